# revision 7
# baseline (speedup 1.0000x reference)
"""MoE layer (top-2 of 8 experts, SwiGLU FFN) on 8 Trainium2 cores.

Strategy: expert-parallel routed compute.
  - The gate (logits -> top-2 -> softmax -> combine weights + lb loss) is
    replicated on the host with the exact same jax ops as the reference so
    the routing decisions match bitwise (a flipped top-2 pick near a tie
    would otherwise produce an O(1) error on that token's output row).
  - Each of the 8 cores runs one expert's SwiGLU FFN over just the tokens
    routed to it (~T*K/E ~ 1100 tokens instead of all 4096), in bf16 on
    the PE array with fp32 PSUM accumulation.
  - Host scatter-adds the 8 scaled partial outputs.
"""

import numpy as np
import ml_dtypes

D = 2048
E = 8
KTOP = 2
H = 5461
HP = 5504  # H padded to 43*128
CD = D // 128  # 16 contraction chunks over D
CH = HP // 128  # 43 contraction chunks over H
B, S = 2, 2048
T = B * S
N_CORES = 8
TT = 512  # token tile per matmul (PSUM bank = 512 fp32)
MAX_SINGLE_BLOCK = 1280  # single-block SBUF budget limit on C

BF16 = ml_dtypes.bfloat16

_prog_cache = {}


def _token_tiles(n):
    """Split n tokens into tiles of <= TT."""
    out = []
    o = 0
    while o < n:
        sz = min(TT, n - o)
        out.append((o, sz))
        o += sz
    return out


def _build_program(C, n_repeat=1):
    """n_repeat>1 wraps the body in a device-side loop (benchmarking only)."""
    import contextlib
    import concourse.mybir as mybir
    import concourse.tile as tile
    from concourse import bacc

    f32 = mybir.dt.float32
    bf16 = mybir.dt.bfloat16

    nc = bacc.Bacc("TRN2", target_bir_lowering=False, debug=False,
                   num_devices=N_CORES)

    xeT = nc.dram_tensor("xeT", [128, CD, C], bf16, kind="ExternalInput").ap()
    w1t = nc.dram_tensor("w1t", [CH, 128, CD * 128], bf16, kind="ExternalInput").ap()
    w3t = nc.dram_tensor("w3t", [CH, 128, CD * 128], bf16, kind="ExternalInput").ap()
    w2t = nc.dram_tensor("w2t", [CD, 128, CH * 128], bf16, kind="ExternalInput").ap()
    gw = nc.dram_tensor("gw", [128, C], f32, kind="ExternalInput").ap()
    yeT = nc.dram_tensor("yeT", [CD, 128, C], f32, kind="ExternalOutput").ap()

    tblk = C if C <= MAX_SINGLE_BLOCK else 1024
    blocks = []
    o = 0
    while o < C:
        sz = min(tblk, C - o)
        blocks.append((o, sz))
        o += sz

    silu = mybir.ActivationFunctionType.Silu

    with tile.TileContext(nc) as tc:
        rep = (tc.For_i(0, n_repeat, 1) if n_repeat > 1
               else contextlib.nullcontext())
        with (
            rep,
            tc.tile_pool(name="xp", bufs=1) as xp,
            tc.tile_pool(name="gp", bufs=1) as gp,
            tc.tile_pool(name="hp", bufs=1) as hp,
            tc.tile_pool(name="wp", bufs=2) as wp,
            tc.tile_pool(name="w2p", bufs=2) as w2p,
            tc.tile_pool(name="sp", bufs=3) as sp,
            tc.tile_pool(name="op", bufs=3) as op,
            tc.tile_pool(name="ps13", bufs=4, space="PSUM") as ps13,
            tc.tile_pool(name="pso", bufs=3, space="PSUM") as pso,
        ):
            w1_first = wp.tile([128, CD, 128], bf16, tag="w1")
            nc.sync.dma_start(w1_first[:], w1t[0])
            w3_first = wp.tile([128, CD, 128], bf16, tag="w3")
            nc.sync.dma_start(w3_first[:], w3t[0])
            x_ch = []
            for c in range(CD):
                xc = xp.tile([128, C], bf16, tag=f"x{c}")
                nc.sync.dma_start(xc[:], xeT[:, c, :])
                x_ch.append(xc)
            gw_sb = gp.tile([128, C], f32)
            nc.sync.dma_start(gw_sb[:], gw[:])

            for b0, bsz in blocks:
                h_sb = hp.tile([128, CH, bsz], bf16, tag="h")
                tts = _token_tiles(bsz)

                # --- GEMM1 & GEMM3 + SwiGLU: h = silu(x@w1.T) * (x@w3.T) ---
                for ht in range(CH):
                    if b0 == 0 and ht == 0:
                        w1_sb, w3_sb = w1_first, w3_first
                    else:
                        w1_sb = wp.tile([128, CD, 128], bf16, tag="w1")
                        nc.sync.dma_start(w1_sb[:], w1t[ht])
                        w3_sb = wp.tile([128, CD, 128], bf16, tag="w3")
                        nc.sync.dma_start(w3_sb[:], w3t[ht])
                    for t0, tsz in tts:
                        g0 = b0 + t0
                        p1 = ps13.tile([128, tsz], f32, tag="p13")
                        for c in range(CD):
                            nc.tensor.matmul(
                                p1[:], w1_sb[:, c, :],
                                x_ch[c][:, g0:g0 + tsz],
                                start=(c == 0), stop=(c == CD - 1))
                        p3 = ps13.tile([128, tsz], f32, tag="p13")
                        for c in range(CD):
                            nc.tensor.matmul(
                                p3[:], w3_sb[:, c, :],
                                x_ch[c][:, g0:g0 + tsz],
                                start=(c == 0), stop=(c == CD - 1))
                        s_sb = sp.tile([128, tsz], f32, tag="silu")
                        nc.scalar.activation(s_sb[:], p1[:], silu)
                        nc.vector.tensor_mul(
                            h_sb[:, ht, t0:t0 + tsz], s_sb[:], p3[:])

                # --- GEMM2: ye = h @ w2.T, scaled by combine weights ---
                for dt in range(CD):
                    w2_sb = w2p.tile([128, CH, 128], bf16, tag="w2")
                    nc.sync.dma_start(w2_sb[:], w2t[dt])
                    for t0, tsz in tts:
                        g0 = b0 + t0
                        po = pso.tile([128, tsz], f32, tag="po")
                        for c in range(CH):
                            nc.tensor.matmul(
                                po[:], w2_sb[:, c, :],
                                h_sb[:, c, t0:t0 + tsz],
                                start=(c == 0), stop=(c == CH - 1))
                        o_sb = op.tile([128, tsz], f32, tag="o")
                        nc.vector.tensor_mul(
                            o_sb[:], po[:], gw_sb[:, g0:g0 + tsz])
                        nc.sync.dma_start(yeT[dt][:, g0:g0 + tsz], o_sb[:])

    nc.compile()
    return nc


def _gate_host(x, wg):
    """Replicate the reference's gating math with the same jax ops."""
    import jax
    import jax.numpy as jnp

    xf = jnp.asarray(x).reshape(-1, D)
    logits = xf @ jnp.asarray(wg).T
    topk_vals, topk_idx = jax.lax.top_k(logits, KTOP)
    topk_probs = jax.nn.softmax(topk_vals, axis=-1)
    gate_probs = jax.nn.softmax(logits, axis=-1)
    mean_probs = gate_probs.mean(axis=0)
    lb_loss = (jnp.std(mean_probs, ddof=1) / (mean_probs.mean() + 1e-10)) ** 2
    cw = jnp.zeros((T, E), xf.dtype).at[
        jnp.arange(T)[:, None], topk_idx].add(topk_probs)
    return (np.asarray(topk_idx), np.asarray(cw), np.asarray(lb_loss))


import os as _os
VERBOSE = _os.environ.get("MOE_KERNEL_VERBOSE", "0") == "1"


def _t(msg, t0):
    import time
    if VERBOSE:
        print(f"[kernel] {msg}: {time.time() - t0:.2f}s", flush=True)
    return time.time()


def kernel(x, wg, w1, w2, w3):
    import time
    t0 = time.time()
    x = np.asarray(x, dtype=np.float32)
    wg = np.asarray(wg, dtype=np.float32)
    w1 = np.asarray(w1, dtype=np.float32)
    w2 = np.asarray(w2, dtype=np.float32)
    w3 = np.asarray(w3, dtype=np.float32)
    assert x.shape == (B, S, D) and w1.shape == (E, H, D)

    topk_idx, cw, lb_loss = _gate_host(x, wg)
    t0 = _t("gate", t0)
    xf = x.reshape(T, D)

    # --- routing: token lists per expert ---
    idx_lists = []
    for e in range(E):
        sel = (topk_idx[:, 0] == e) | (topk_idx[:, 1] == e)
        idx_lists.append(np.nonzero(sel)[0])
    max_cnt = max(len(ix) for ix in idx_lists)
    C = max(128, -(-max_cnt // 8) * 8)

    # --- per-core device inputs ---
    in_maps = []
    for e in range(E):
        ix = idx_lists[e]
        cnt = len(ix)
        xe = np.zeros((C, D), np.float32)
        xe[:cnt] = xf[ix]
        # [p, c, n] = xe.T[c*128+p, n]
        xeT = np.ascontiguousarray(
            xe.T.reshape(CD, 128, C).transpose(1, 0, 2)).astype(BF16)

        w1p = np.zeros((HP, D), np.float32)
        w1p[:H] = w1[e]
        w1tt = np.ascontiguousarray(
            w1p.reshape(CH, 128, CD, 128).transpose(0, 3, 2, 1)
        ).reshape(CH, 128, CD * 128).astype(BF16)

        w3p = np.zeros((HP, D), np.float32)
        w3p[:H] = w3[e]
        w3tt = np.ascontiguousarray(
            w3p.reshape(CH, 128, CD, 128).transpose(0, 3, 2, 1)
        ).reshape(CH, 128, CD * 128).astype(BF16)

        w2p = np.zeros((D, HP), np.float32)
        w2p[:, :H] = w2[e]
        w2tt = np.ascontiguousarray(
            w2p.reshape(CD, 128, CH, 128).transpose(0, 3, 2, 1)
        ).reshape(CD, 128, CH * 128).astype(BF16)

        gwv = np.zeros((C,), np.float32)
        gwv[:cnt] = cw[ix, e]
        gwb = np.ascontiguousarray(
            np.broadcast_to(gwv[None, :], (128, C)))

        in_maps.append({"xeT": xeT, "w1t": w1tt, "w3t": w3tt,
                        "w2t": w2tt, "gw": gwb})
    t0 = _t("host prep", t0)

    # --- compile (cached) + run on 8 cores ---
    from concourse.bass_utils import run_bass_kernel_spmd

    if C not in _prog_cache:
        _prog_cache[C] = _build_program(C)
    nc = _prog_cache[C]
    t0 = _t("build+compile", t0)

    res = run_bass_kernel_spmd(nc, in_maps, core_ids=list(range(N_CORES)))
    t0 = _t("device run", t0)

    # --- combine: scatter-add scaled expert outputs ---
    out = np.zeros((T, D), np.float32)
    for e in range(E):
        ix = idx_lists[e]
        cnt = len(ix)
        ye = res.results[e]["yeT"].reshape(D, C)
        out[ix] += ye[:, :cnt].T
    t0 = _t("combine", t0)

    return out.reshape(B, S, D), np.float32(0.01) * lb_loss.astype(np.float32)
